# revision 59
# baseline (speedup 1.0000x reference)
"""DCGRU cell Trainium2 kernel (v4).

Math (per batch i):
  xs = [input, state]                                  [N, 66]
  aggr[j] = S[j] @ xs          (J=4 supports)          [N, 66]
  r = sigmoid(sum_j aggr[j] @ Wr[j] + br)              [N, 64]
  u = sigmoid(sum_j aggr[j] @ Wu[j] + bu)
  xc = [input, r*state]
  c = tanh(sum_j (S[j] @ xc) @ Wc[j] + bc)             (bc == 0 per spec)
  out = u*state + (1-u)*c

Sharding: data-parallel over batch, 8 batches per core on 8 cores.
supports/weights replicated. No collectives.

Device layout (per core, Bc=8), all matmul operands fp16.

One software-pipelined loop over 8 k-groups of 256 runs four stages,
each trailing the previous by one iteration so PE never waits:
  big(g):  aggr[j] = S[j] @ xs in [k, (i,f)] psum, 16 m-block
           accumulation chains (moving xs SBUF-resident, stationary ST
           row-blocks streamed on SP), drained fp16 on DVE/Act.
  epi(g):  PE-transpose agg -> [f, k] slices, W-project (contract f,
           j-accumulated in psum), sigmoid+bias -> ruT[i,g] fp16
           [128=(r|u), 256].
  late(g): u.T -> uT[kb] [k, (i,o)] PE transposes; xcT[i] rows 0:64
           k-slice = r * state.T (DVE mul vs streamed state.T slice);
           then y[j, mb in g] = xcT.T @ Wc[j] [128m, (i,o)] via
           66-contraction matmuls into j-paired fp16 psum.

Phase 2 (c) per k-block kb: c_pre = sum_{j,mb} ST[j,mb,kb] @ y[j,mb]:
one 64-matmul psum chain of 512 cols (stationary stK tiles streamed);
tanh -> c fp16 [k, (i,o)]; GRU combine (3 DVE ops vs uT[kb] and a
streamed state [k,(i,o)] tile) and the output DMA ride along each
k-block -> no serial tail.

xc rows are [state(0:64), input(64:66)] (input.T DMA'd once at start);
Wc rows are reordered host-side to match.
"""

import sys

if '/opt/trn_rl_repo' not in sys.path:
    sys.path.insert(0, '/opt/trn_rl_repo')

import numpy as np

B, N, IN, OUT, J = 64, 2048, 2, 64, 4
NCORES = 8
BC = B // NCORES            # 8 batches per core
F = IN + OUT                # 66
CB = BC * F                 # 528 moving columns
P = 128
HALF = CB // 2              # 264 (psum bank split)
NMB = N // P                # 16 m blocks
NKB = N // P                # 16 k blocks
KBG = 2                     # k blocks per psum group
NG = NKB // KBG             # 8 groups
MBQ = 8                     # m blocks per ST dma
CO = BC * OUT               # 512 combine columns
GW = KBG * P                # 256 group width

_CACHE = {}


def _build_module():
    import concourse.tile as tile
    import concourse.mybir as mybir
    from concourse import bacc
    from concourse.masks import make_identity

    f32 = mybir.dt.float32
    f16 = mybir.dt.float16
    AF = mybir.ActivationFunctionType

    nc = bacc.Bacc("TRN2", target_bir_lowering=False, debug=False,
                   num_devices=1)

    st_d = nc.dram_tensor("st", [J, N, N], f16, kind="ExternalInput").ap()
    stK_d = nc.dram_tensor("stK", [J, NKB, P, NMB, P], f16,
                           kind="ExternalInput").ap()
    xs_d = nc.dram_tensor("xs", [N, CB], f16, kind="ExternalInput").ap()
    xinTg_d = nc.dram_tensor("xinTg", [NG, IN, BC, GW], f16,
                             kind="ExternalInput").ap()
    stTg_d = nc.dram_tensor("stTg", [NG, OUT, BC, GW], f16,
                            kind="ExternalInput").ap()
    stateK_d = nc.dram_tensor("stateK", [NKB, P, CO], f16,
                              kind="ExternalInput").ap()
    wru_d = nc.dram_tensor("wru", [J, F, 2 * OUT], f16,
                           kind="ExternalInput").ap()
    wc_d = nc.dram_tensor("wc", [J // 2, F, 2 * OUT], f16,
                          kind="ExternalInput").ap()
    bru_d = nc.dram_tensor("bru", [2 * OUT, 1], f32, kind="ExternalInput").ap()
    out_d = nc.dram_tensor("out", [NKB, P, CO], f16,
                           kind="ExternalOutput").ap()

    with tile.TileContext(nc) as tc:
        with tc.tile_pool(name="const", bufs=1) as const_pool, \
             tc.tile_pool(name="uT", bufs=NKB) as uT_pool, \
             tc.tile_pool(name="stK0", bufs=4) as stK0_pool, \
             tc.tile_pool(name="y", bufs=2 * NMB) as y_pool:

            uT_tiles = [uT_pool.tile([P, CO], f16, tag="uT", name=f"uT{kb}")
                        for kb in range(NKB)]
            # y[jp][mb] holds j = 2*jp (cols 0:CO) and j = 2*jp+1 (CO:2CO)
            y_tiles = [[y_pool.tile([P, 2 * CO], f16, tag="y",
                                    name=f"y{jp}_{mb}")
                        for mb in range(NMB)] for jp in range(2)]

            ident = const_pool.tile([P, P], f16, tag="ident")
            make_identity(nc, ident[:])
            # consts on the Pool SWDGE queue: no HWDGE contention with
            # the SP-side ST stream at startup
            wru_t = []
            wc_t = []
            for j in range(J):
                w1 = const_pool.tile([F, 2 * OUT], f16, tag=f"wru{j}")
                nc.gpsimd.dma_start(w1[:], wru_d[j])
                wru_t.append(w1)
                if j < J // 2:
                    w2 = const_pool.tile([F, 2 * OUT], f16, tag=f"wc{j}")
                    nc.gpsimd.dma_start(w2[:], wc_d[j])
                    wc_t.append(w2)
            bru_t = const_pool.tile([2 * OUT, 1], f32, tag="bru")
            nc.gpsimd.dma_start(bru_t[:], bru_d[:])

            # ---------------- phase 1 (pipelined) ----------------
            with tc.tile_pool(name="xs", bufs=NMB) as xs_pool, \
                 tc.tile_pool(name="xcg", bufs=2) as xcg_pool, \
                 tc.tile_pool(name="ruTg", bufs=3 * BC) as ruT_pool, \
                 tc.tile_pool(name="stTg", bufs=2) as stTg_pool, \
                 tc.tile_pool(name="stst", bufs=8) as st_pool, \
                 tc.tile_pool(name="agg", bufs=2 * J * KBG) as agg_pool, \
                 tc.tile_pool(name="aggT", bufs=34) as aggT_pool, \
                 tc.tile_pool(name="aggps", bufs=4, space="PSUM") as agg_ps_pool, \
                 tc.tile_pool(name="tpps", bufs=4, space="PSUM") as tp_ps_pool:
                y_ps_pool = agg_ps_pool
                ps_ctr = [0]

                agg_sb = {}
                xcg_tiles = {}
                stK0_tiles = []
                ru_g = {}
                stTg_tiles = {}
                st_tiles = {}

                def issue_st(g, j, splits=(MBQ, MBQ)):
                    k0 = g * GW
                    ts = []
                    mb0 = 0
                    for cnt in splits:
                        st_t = st_pool.tile([P, cnt, GW], f16, tag="st",
                                            name=f"st{cnt}")
                        src = st_d[j, mb0 * P:(mb0 + cnt) * P,
                                   k0:k0 + GW]
                        src = src.rearrange("(g p) k -> p g k", p=P)
                        nc.sync.dma_start(st_t[:], src)
                        ts.append((st_t, mb0, cnt))
                        mb0 += cnt
                    st_tiles[(g, j)] = ts

                # startup: first ST tiles ahead of the xs loads so the
                # first accumulation chain starts ASAP
                xs_tiles = []
                for mb in range(NMB):
                    t = xs_pool.tile([P, CB], f16, tag="xs")
                    xs_tiles.append(t)
                issue_st(0, 0, splits=(2, 6, 8))
                for mb in range(NMB):
                    nc.sync.dma_start(xs_tiles[mb][:],
                                      xs_d[mb * P:(mb + 1) * P, :])
                for j in range(1, J):
                    issue_st(0, j)

                def big_mm(g):
                    for j in range(J):
                        if (g, j) not in st_tiles:
                            issue_st(g, j)
                        st_ts = st_tiles[(g, j)]
                        for kb in range(KBG):
                            t = agg_pool.tile([P, CB], f16, tag="agg",
                                              name=f"agg{g % 2}_{j}_{kb}")
                            agg_sb[(g % 2, j, kb)] = t
                            for h in range(2):
                                ps_ctr[0] += 1
                                pst = agg_ps_pool.tile(
                                    [P, HALF], f32, tag="aggps",
                                    name=f"aggps{ps_ctr[0] % 4}")
                                for mb in range(NMB):
                                    for st_t, mb0, cnt in st_ts:
                                        if mb0 <= mb < mb0 + cnt:
                                            break
                                    lhsT = st_t[:, mb - mb0,
                                                kb * P:(kb + 1) * P]
                                    nc.tensor.matmul(
                                        pst[:],
                                        lhsT,
                                        xs_tiles[mb][:, h * HALF:(h + 1) * HALF],
                                        start=(mb == 0),
                                        stop=(mb == NMB - 1),
                                    )
                                if (kb + h) % 2 == 0:
                                    nc.vector.tensor_copy(
                                        t[:, h * HALF:(h + 1) * HALF], pst[:])
                                else:
                                    nc.scalar.copy(
                                        t[:, h * HALF:(h + 1) * HALF], pst[:])
                        st_tiles.pop((g, j))

                def epi(g):
                    # state.T slice for group g, consumed by late(g) next
                    # iteration
                    stg = stTg_pool.tile([OUT, BC, GW], f16, tag="stTg")
                    nc.sync.dma_start(stg[:], stTg_d[g])
                    stTg_tiles[g] = stg
                    xcg = xcg_pool.tile([F, BC, GW], f16, tag="xcg")
                    nc.gpsimd.dma_start(xcg[OUT:F, :, :], xinTg_d[g])
                    xcg_tiles[g] = xcg
                    if g >= NG - 2:
                        # phase-2 kb=0 stationary tiles, prefetched while
                        # SP is otherwise idle
                        for j in range(2):
                            st_t = stK0_pool.tile([P, NMB, P], f16,
                                                  tag="stK0")
                            jj = 2 * (g - NG + 2) + j
                            nc.sync.dma_start(st_t[:], stK_d[jj, 0])
                            stK0_tiles.append(st_t)
                    aggT = {}
                    for i in range(BC):
                        for j in range(J):
                            tp = tp_ps_pool.tile([F, GW], f16, tag="tpps",
                                                 name=f"tp{(2 * i + j) % 3}")
                            for kb in range(KBG):
                                nc.tensor.transpose(
                                    tp[:, kb * P:(kb + 1) * P],
                                    agg_sb[(g % 2, j, kb)]
                                    [:, i * F:(i + 1) * F],
                                    ident[:])
                            at = aggT_pool.tile([F, GW], f16, tag="aggT",
                                                name=f"aggT{i}_{j}")
                            if (i + j) % 2 == 0:
                                nc.vector.tensor_copy(at[:], tp[:])
                            else:
                                nc.scalar.copy(at[:], tp[:])
                            aggT[(i, j)] = at
                    for i in range(BC):
                        pp = tp_ps_pool.tile([2 * OUT, GW], f32,
                                             tag="tpps", name=f"proj{i % 2}")
                        for j in range(J):
                            nc.tensor.matmul(
                                pp[:], wru_t[j][:], aggT[(i, j)][:],
                                start=(j == 0), stop=(j == J - 1))
                        rut = ruT_pool.tile([P, GW], f16, tag="ruTg",
                                            name=f"ruT{i}_{g % 3}")
                        nc.scalar.activation(rut[:], pp[:],
                                             AF.Sigmoid, bias=bru_t[:, 0:1])
                        ru_g[(g, i)] = rut

                def late(g):
                    k0 = g * GW
                    # u.T tiles for the phase-2 combine
                    for kb in range(g * KBG, (g + 1) * KBG):
                        c0 = (kb % KBG) * P
                        ups = tp_ps_pool.tile([P, CO], f16, tag="tpps",
                                              name=f"ut{kb % 2}")
                        for i in range(BC):
                            nc.tensor.transpose(
                                ups[:, i * OUT:(i + 1) * OUT],
                                ru_g[(g, i)][OUT:2 * OUT, c0:c0 + P],
                                ident[OUT:P, OUT:P])
                        if kb % 2 == 0:
                            nc.vector.tensor_copy(uT_tiles[kb][:], ups[:])
                        else:
                            nc.scalar.copy(uT_tiles[kb][:], ups[:])
                    # xcg rows 0:64 = (r*state).T for this k range
                    stg = stTg_tiles.pop(g)
                    xcg = xcg_tiles[g]
                    for i in range(BC):
                        nc.vector.tensor_mul(
                            xcg[0:OUT, i, :],
                            ru_g[(g, i)][0:OUT, :],
                            stg[:, i, :])
                        ru_g.pop((g, i))
                    # y pre-projection for this group's two m blocks
                    for mb in range(g * KBG, (g + 1) * KBG):
                        c0 = (mb % KBG) * P
                        for jp in range(2):
                            for ih in range(2):
                                ps_ctr[0] += 1
                                yps = y_ps_pool.tile(
                                    [P, CO], f32, tag="aggps",
                                    name=f"aggps{ps_ctr[0] % 4}")
                                for ii in range(BC // 2):
                                    i = ih * (BC // 2) + ii
                                    nc.tensor.matmul(
                                        yps[:, ii * 2 * OUT:
                                            (ii + 1) * 2 * OUT],
                                        xcg[:, i, c0:c0 + P],
                                        wc_t[jp][:],
                                        start=True, stop=True)
                                # psum cols (ii, jh, o) -> y cols (jh, i, o)
                                srcv = yps[:].rearrange(
                                    "p (ii jh o) -> p jh ii o", jh=2, o=OUT)
                                dstv = y_tiles[jp][mb][:].rearrange(
                                    "p (jh i o) -> p jh i o", jh=2, o=OUT)
                                dstv = dstv[:, :, ih * (BC // 2):
                                            (ih + 1) * (BC // 2), :]
                                if (mb + jp + ih) % 2 == 0:
                                    nc.vector.tensor_copy(dstv, srcv)
                                else:
                                    nc.scalar.copy(dstv, srcv)

                for it in range(NG + 2):
                    if it < NG:
                        big_mm(it)
                    if 1 <= it <= NG:
                        epi(it - 1)
                    if 2 <= it:
                        late(it - 2)

            # ---------------- phase 2: diffusion + combine ----------
            with tc.tile_pool(name="stK", bufs=12) as stK_pool, \
                 tc.tile_pool(name="stv", bufs=3) as stv_pool, \
                 tc.tile_pool(name="cmb", bufs=8) as cmb_pool, \
                 tc.tile_pool(name="cps", bufs=2, space="PSUM") as c_ps_pool:
                for kb in range(NKB):
                    if kb == 0:
                        stk_ts = stK0_tiles
                    else:
                        stk_ts = []
                        for j in range(J):
                            st_t = stK_pool.tile([P, NMB, P], f16, tag="stK")
                            if kb == 1:
                                nc.scalar.dma_start(st_t[:], stK_d[j, kb])
                            else:
                                nc.sync.dma_start(st_t[:], stK_d[j, kb])
                            stk_ts.append(st_t)
                    stv = stv_pool.tile([P, CO], f16, tag="stv")
                    nc.scalar.dma_start(stv[:], stateK_d[kb])
                    cps = c_ps_pool.tile([P, CO], f32, tag="cps",
                                         name=f"cps{kb % 2}")
                    for mb in range(NMB):
                        for j in range(J):
                            nc.tensor.matmul(
                                cps[:],
                                stk_ts[j][:, mb, :],
                                y_tiles[j // 2][mb][:, (j % 2) * CO:
                                                    (j % 2 + 1) * CO],
                                start=(mb == 0 and j == 0),
                                stop=(mb == NMB - 1 and j == J - 1))
                    ct = cmb_pool.tile([P, CO], f16, tag="cmb",
                                       name=f"c{kb % 2}")
                    nc.scalar.activation(ct[:], cps[:], AF.Tanh)
                    # out = c + u*(state - c)
                    t1 = cmb_pool.tile([P, CO], f16, tag="cmb",
                                       name=f"t1_{kb % 2}")
                    nc.vector.tensor_sub(t1[:], stv[:], ct[:])
                    t2 = cmb_pool.tile([P, CO], f16, tag="cmb",
                                       name=f"t2_{kb % 2}")
                    nc.vector.tensor_mul(t2[:], uT_tiles[kb][:], t1[:])
                    t3 = cmb_pool.tile([P, CO], f16, tag="cmb",
                                       name=f"t3_{kb % 2}")
                    nc.vector.tensor_add(t3[:], ct[:], t2[:])
                    nc.scalar.dma_start(out_d[kb], t3[:])

    nc.compile()
    return nc


def _get_module():
    if "nc" not in _CACHE:
        _CACHE["nc"] = _build_module()
    return _CACHE["nc"]


def kernel(input, state, supports, Wr, br, Wu, bu, Wc, bc):
    input = np.asarray(input, np.float32)
    state = np.asarray(state, np.float32)
    supports = np.asarray(supports, np.float32)
    Wr = np.asarray(Wr, np.float32)
    br = np.asarray(br, np.float32)
    Wu = np.asarray(Wu, np.float32)
    bu = np.asarray(bu, np.float32)
    Wc = np.asarray(Wc, np.float32)
    bc = np.asarray(bc, np.float32)

    assert np.all(bc == 0.0), "kernel assumes bc == 0 (spec fill: zeros)"

    from concourse.bass_utils import run_bass_kernel_spmd

    nc = _get_module()

    f16 = np.float16
    st_host = np.ascontiguousarray(supports.transpose(0, 2, 1).astype(f16))
    stK_host = np.ascontiguousarray(
        st_host.reshape(J, NMB, P, NKB, P).transpose(0, 3, 2, 1, 4))
    wru = np.ascontiguousarray(np.concatenate([Wr, Wu], axis=2).astype(f16))
    # xc rows are [state(0:64), input(64:66)] on device; reorder Wc to match
    wc_r = np.concatenate([Wc[:, IN:, :], Wc[:, :IN, :]], axis=1)
    wc_host = np.ascontiguousarray(
        wc_r.reshape(J // 2, 2, F, OUT).transpose(0, 2, 1, 3)
        .reshape(J // 2, F, 2 * OUT).astype(f16))
    bru = np.concatenate([br, bu]).reshape(2 * OUT, 1).astype(np.float32)
    xs_full = np.concatenate([input, state], axis=2)  # [B, N, F]

    in_maps = []
    for c in range(NCORES):
        sl = slice(c * BC, (c + 1) * BC)
        xs_c = np.ascontiguousarray(
            xs_full[sl].transpose(1, 0, 2).reshape(N, CB).astype(f16))
        xinTg_c = np.ascontiguousarray(
            input[sl].astype(f16).reshape(BC, NG, GW, IN)
            .transpose(1, 3, 0, 2))
        st16 = state[sl].astype(f16)                   # [BC, N, OUT]
        stTg_c = np.ascontiguousarray(
            st16.reshape(BC, NG, GW, OUT).transpose(1, 3, 0, 2))
        stateK_c = np.ascontiguousarray(
            st16.reshape(BC, NKB, P, OUT).transpose(1, 2, 0, 3)
            .reshape(NKB, P, CO))
        in_maps.append({
            "st": st_host,
            "stK": stK_host,
            "xs": xs_c,
            "xinTg": xinTg_c,
            "stTg": stTg_c,
            "stateK": stateK_c,
            "wru": wru,
            "wc": wc_host,
            "bru": bru,
        })

    import time
    t0 = time.monotonic()
    res = run_bass_kernel_spmd(nc, in_maps, core_ids=list(range(NCORES)))
    _CACHE["last_wall_s"] = time.monotonic() - t0

    out = np.empty((B, N, OUT), np.float32)
    for c in range(NCORES):
        o2 = res.results[c]["out"]              # [NKB, P, BC*OUT] fp16
        o2 = o2.reshape(NKB, P, BC, OUT).transpose(2, 0, 1, 3)
        out[c * BC:(c + 1) * BC] = o2.reshape(BC, N, OUT).astype(np.float32)
    return out


# revision 61
# speedup vs baseline: 1.0018x; 1.0018x over previous
"""DCGRU cell Trainium2 kernel (v4).

Math (per batch i):
  xs = [input, state]                                  [N, 66]
  aggr[j] = S[j] @ xs          (J=4 supports)          [N, 66]
  r = sigmoid(sum_j aggr[j] @ Wr[j] + br)              [N, 64]
  u = sigmoid(sum_j aggr[j] @ Wu[j] + bu)
  xc = [input, r*state]
  c = tanh(sum_j (S[j] @ xc) @ Wc[j] + bc)             (bc == 0 per spec)
  out = u*state + (1-u)*c

Sharding: data-parallel over batch, 8 batches per core on 8 cores.
supports/weights replicated. No collectives.

Device layout (per core, Bc=8), all matmul operands fp16.

One software-pipelined loop over 8 k-groups of 256 runs four stages,
each trailing the previous by one iteration so PE never waits:
  big(g):  aggr[j] = S[j] @ xs in [k, (i,f)] psum, 16 m-block
           accumulation chains (moving xs SBUF-resident, stationary ST
           row-blocks streamed on SP), drained fp16 on DVE/Act.
  epi(g):  PE-transpose agg -> [f, k] slices, W-project (contract f,
           j-accumulated in psum), sigmoid+bias -> ruT[i,g] fp16
           [128=(r|u), 256].
  late(g): u.T -> uT[kb] [k, (i,o)] PE transposes; xcT[i] rows 0:64
           k-slice = r * state.T (DVE mul vs streamed state.T slice);
           then y[j, mb in g] = xcT.T @ Wc[j] [128m, (i,o)] via
           66-contraction matmuls into j-paired fp16 psum.

Phase 2 (c) per k-block kb: c_pre = sum_{j,mb} ST[j,mb,kb] @ y[j,mb]:
one 64-matmul psum chain of 512 cols (stationary stK tiles streamed);
tanh -> c fp16 [k, (i,o)]; GRU combine (3 DVE ops vs uT[kb] and a
streamed state [k,(i,o)] tile) and the output DMA ride along each
k-block -> no serial tail.

xc rows are [state(0:64), input(64:66)] (input.T DMA'd once at start);
Wc rows are reordered host-side to match.
"""

import sys

if '/opt/trn_rl_repo' not in sys.path:
    sys.path.insert(0, '/opt/trn_rl_repo')

import numpy as np

B, N, IN, OUT, J = 64, 2048, 2, 64, 4
NCORES = 8
BC = B // NCORES            # 8 batches per core
F = IN + OUT                # 66
CB = BC * F                 # 528 moving columns
P = 128
HALF = CB // 2              # 264 (psum bank split)
NMB = N // P                # 16 m blocks
NKB = N // P                # 16 k blocks
KBG = 2                     # k blocks per psum group
NG = NKB // KBG             # 8 groups
MBQ = 8                     # m blocks per ST dma
CO = BC * OUT               # 512 combine columns
GW = KBG * P                # 256 group width

_CACHE = {}


def _build_module():
    import concourse.tile as tile
    import concourse.mybir as mybir
    from concourse import bacc
    from concourse.masks import make_identity

    f32 = mybir.dt.float32
    f16 = mybir.dt.float16
    AF = mybir.ActivationFunctionType

    nc = bacc.Bacc("TRN2", target_bir_lowering=False, debug=False,
                   num_devices=1)

    st_d = nc.dram_tensor("st", [J, N, N], f16, kind="ExternalInput").ap()
    stK_d = nc.dram_tensor("stK", [J, NKB, P, NMB, P], f16,
                           kind="ExternalInput").ap()
    xs_d = nc.dram_tensor("xs", [N, CB], f16, kind="ExternalInput").ap()
    xinTg_d = nc.dram_tensor("xinTg", [NG, IN, BC, GW], f16,
                             kind="ExternalInput").ap()
    stTg_d = nc.dram_tensor("stTg", [NG, OUT, BC, GW], f16,
                            kind="ExternalInput").ap()
    stateK_d = nc.dram_tensor("stateK", [NKB, P, CO], f16,
                              kind="ExternalInput").ap()
    wru_d = nc.dram_tensor("wru", [J, F, 2 * OUT], f16,
                           kind="ExternalInput").ap()
    wc_d = nc.dram_tensor("wc", [J // 2, F, 2 * OUT], f16,
                          kind="ExternalInput").ap()
    bru_d = nc.dram_tensor("bru", [2 * OUT, 1], f32, kind="ExternalInput").ap()
    out_d = nc.dram_tensor("out", [NKB, P, CO], f16,
                           kind="ExternalOutput").ap()

    with tile.TileContext(nc) as tc:
        with tc.tile_pool(name="const", bufs=1) as const_pool, \
             tc.tile_pool(name="uT", bufs=NKB) as uT_pool, \
             tc.tile_pool(name="stK0", bufs=4) as stK0_pool, \
             tc.tile_pool(name="y", bufs=2 * NMB) as y_pool:

            uT_tiles = [uT_pool.tile([P, CO], f16, tag="uT", name=f"uT{kb}")
                        for kb in range(NKB)]
            # y[jp][mb] holds j = 2*jp (cols 0:CO) and j = 2*jp+1 (CO:2CO)
            y_tiles = [[y_pool.tile([P, 2 * CO], f16, tag="y",
                                    name=f"y{jp}_{mb}")
                        for mb in range(NMB)] for jp in range(2)]

            ident = const_pool.tile([P, P], f16, tag="ident")
            make_identity(nc, ident[:])
            # consts on the Pool SWDGE queue: no HWDGE contention with
            # the SP-side ST stream at startup
            wru_t = []
            wc_t = []
            for j in range(J):
                w1 = const_pool.tile([F, 2 * OUT], f16, tag=f"wru{j}")
                nc.gpsimd.dma_start(w1[:], wru_d[j])
                wru_t.append(w1)
                if j < J // 2:
                    w2 = const_pool.tile([F, 2 * OUT], f16, tag=f"wc{j}")
                    nc.gpsimd.dma_start(w2[:], wc_d[j])
                    wc_t.append(w2)
            bru_t = const_pool.tile([2 * OUT, 1], f32, tag="bru")
            nc.gpsimd.dma_start(bru_t[:], bru_d[:])

            # ---------------- phase 1 (pipelined) ----------------
            with tc.tile_pool(name="xs", bufs=NMB) as xs_pool, \
                 tc.tile_pool(name="xcg", bufs=2) as xcg_pool, \
                 tc.tile_pool(name="ruTg", bufs=3 * BC) as ruT_pool, \
                 tc.tile_pool(name="stTg", bufs=2) as stTg_pool, \
                 tc.tile_pool(name="stst", bufs=8) as st_pool, \
                 tc.tile_pool(name="agg", bufs=2 * J * KBG) as agg_pool, \
                 tc.tile_pool(name="aggT", bufs=34) as aggT_pool, \
                 tc.tile_pool(name="aggps", bufs=4, space="PSUM") as agg_ps_pool, \
                 tc.tile_pool(name="tpps", bufs=4, space="PSUM") as tp_ps_pool:
                y_ps_pool = agg_ps_pool
                ps_ctr = [0]

                agg_sb = {}
                xcg_tiles = {}
                stK0_tiles = []
                ru_g = {}
                stTg_tiles = {}
                st_tiles = {}

                def issue_st(g, j, mbq=MBQ):
                    k0 = g * GW
                    ts = []
                    for mq in range(NMB // mbq):
                        st_t = st_pool.tile([P, mbq, GW], f16, tag="st",
                                            name=f"st{mbq}")
                        src = st_d[j, mq * mbq * P:(mq + 1) * mbq * P,
                                   k0:k0 + GW]
                        src = src.rearrange("(g p) k -> p g k", p=P)
                        nc.sync.dma_start(st_t[:], src)
                        ts.append(st_t)
                    st_tiles[(g, j)] = (ts, mbq)

                # startup: first ST tiles ahead of the xs loads so the
                # first accumulation chain starts ASAP
                xs_tiles = []
                for mb in range(NMB):
                    t = xs_pool.tile([P, CB], f16, tag="xs")
                    xs_tiles.append(t)
                issue_st(0, 0)
                for mb in range(NMB):
                    nc.sync.dma_start(xs_tiles[mb][:],
                                      xs_d[mb * P:(mb + 1) * P, :])
                for j in range(1, J):
                    issue_st(0, j)

                def big_mm(g):
                    for j in range(J):
                        if (g, j) not in st_tiles:
                            issue_st(g, j)
                        st_ts, mbq = st_tiles[(g, j)]
                        for kb in range(KBG):
                            t = agg_pool.tile([P, CB], f16, tag="agg",
                                              name=f"agg{g % 2}_{j}_{kb}")
                            agg_sb[(g % 2, j, kb)] = t
                            for h in range(2):
                                ps_ctr[0] += 1
                                pst = agg_ps_pool.tile(
                                    [P, HALF], f32, tag="aggps",
                                    name=f"aggps{ps_ctr[0] % 4}")
                                for mb in range(NMB):
                                    mq, ml = divmod(mb, mbq)
                                    lhsT = st_ts[mq][:, ml,
                                                     kb * P:(kb + 1) * P]
                                    nc.tensor.matmul(
                                        pst[:],
                                        lhsT,
                                        xs_tiles[mb][:, h * HALF:(h + 1) * HALF],
                                        start=(mb == 0),
                                        stop=(mb == NMB - 1),
                                    )
                                if (kb + h) % 2 == 0:
                                    nc.vector.tensor_copy(
                                        t[:, h * HALF:(h + 1) * HALF], pst[:])
                                else:
                                    nc.scalar.copy(
                                        t[:, h * HALF:(h + 1) * HALF], pst[:])
                        st_tiles.pop((g, j))

                def epi(g):
                    # state.T slice for group g, consumed by late(g) next
                    # iteration
                    stg = stTg_pool.tile([OUT, BC, GW], f16, tag="stTg")
                    nc.sync.dma_start(stg[:], stTg_d[g])
                    stTg_tiles[g] = stg
                    xcg = xcg_pool.tile([F, BC, GW], f16, tag="xcg")
                    nc.gpsimd.dma_start(xcg[OUT:F, :, :], xinTg_d[g])
                    xcg_tiles[g] = xcg
                    if g >= NG - 2:
                        # phase-2 kb=0 stationary tiles, prefetched while
                        # SP is otherwise idle
                        for j in range(2):
                            st_t = stK0_pool.tile([P, NMB, P], f16,
                                                  tag="stK0")
                            jj = 2 * (g - NG + 2) + j
                            nc.sync.dma_start(st_t[:], stK_d[jj, 0])
                            stK0_tiles.append(st_t)
                    aggT = {}
                    for i in range(BC):
                        for j in range(J):
                            tp = tp_ps_pool.tile([F, GW], f16, tag="tpps",
                                                 name=f"tp{(2 * i + j) % 3}")
                            for kb in range(KBG):
                                nc.tensor.transpose(
                                    tp[:, kb * P:(kb + 1) * P],
                                    agg_sb[(g % 2, j, kb)]
                                    [:, i * F:(i + 1) * F],
                                    ident[:])
                            at = aggT_pool.tile([F, GW], f16, tag="aggT",
                                                name=f"aggT{i}_{j}")
                            if (i + j) % 2 == 0:
                                nc.vector.tensor_copy(at[:], tp[:])
                            else:
                                nc.scalar.copy(at[:], tp[:])
                            aggT[(i, j)] = at
                    for i in range(BC):
                        pp = tp_ps_pool.tile([2 * OUT, GW], f32,
                                             tag="tpps", name=f"proj{i % 2}")
                        for j in range(J):
                            nc.tensor.matmul(
                                pp[:], wru_t[j][:], aggT[(i, j)][:],
                                start=(j == 0), stop=(j == J - 1))
                        rut = ruT_pool.tile([P, GW], f16, tag="ruTg",
                                            name=f"ruT{i}_{g % 3}")
                        nc.scalar.activation(rut[:], pp[:],
                                             AF.Sigmoid, bias=bru_t[:, 0:1])
                        ru_g[(g, i)] = rut

                def late(g):
                    k0 = g * GW
                    # u.T tiles for the phase-2 combine
                    for kb in range(g * KBG, (g + 1) * KBG):
                        c0 = (kb % KBG) * P
                        ups = tp_ps_pool.tile([P, CO], f16, tag="tpps",
                                              name=f"ut{kb % 2}")
                        for i in range(BC):
                            nc.tensor.transpose(
                                ups[:, i * OUT:(i + 1) * OUT],
                                ru_g[(g, i)][OUT:2 * OUT, c0:c0 + P],
                                ident[OUT:P, OUT:P])
                        if kb % 2 == 0:
                            nc.vector.tensor_copy(uT_tiles[kb][:], ups[:])
                        else:
                            nc.scalar.copy(uT_tiles[kb][:], ups[:])
                    # xcg rows 0:64 = (r*state).T for this k range
                    stg = stTg_tiles.pop(g)
                    xcg = xcg_tiles[g]
                    for i in range(BC):
                        nc.vector.tensor_mul(
                            xcg[0:OUT, i, :],
                            ru_g[(g, i)][0:OUT, :],
                            stg[:, i, :])
                        ru_g.pop((g, i))
                    # y pre-projection for this group's two m blocks
                    for mb in range(g * KBG, (g + 1) * KBG):
                        c0 = (mb % KBG) * P
                        for jp in range(2):
                            for ih in range(2):
                                ps_ctr[0] += 1
                                yps = y_ps_pool.tile(
                                    [P, CO], f32, tag="aggps",
                                    name=f"aggps{ps_ctr[0] % 4}")
                                for ii in range(BC // 2):
                                    i = ih * (BC // 2) + ii
                                    nc.tensor.matmul(
                                        yps[:, ii * 2 * OUT:
                                            (ii + 1) * 2 * OUT],
                                        xcg[:, i, c0:c0 + P],
                                        wc_t[jp][:],
                                        start=True, stop=True)
                                # psum cols (ii, jh, o) -> y cols (jh, i, o)
                                srcv = yps[:].rearrange(
                                    "p (ii jh o) -> p jh ii o", jh=2, o=OUT)
                                dstv = y_tiles[jp][mb][:].rearrange(
                                    "p (jh i o) -> p jh i o", jh=2, o=OUT)
                                dstv = dstv[:, :, ih * (BC // 2):
                                            (ih + 1) * (BC // 2), :]
                                if (mb + jp + ih) % 2 == 0:
                                    nc.vector.tensor_copy(dstv, srcv)
                                else:
                                    nc.scalar.copy(dstv, srcv)

                for it in range(NG + 2):
                    if it < NG:
                        big_mm(it)
                    if 1 <= it <= NG:
                        epi(it - 1)
                    if 2 <= it:
                        late(it - 2)

            # ---------------- phase 2: diffusion + combine ----------
            with tc.tile_pool(name="stK", bufs=12) as stK_pool, \
                 tc.tile_pool(name="stv", bufs=3) as stv_pool, \
                 tc.tile_pool(name="cmb", bufs=8) as cmb_pool, \
                 tc.tile_pool(name="cps", bufs=2, space="PSUM") as c_ps_pool:
                for kb in range(NKB):
                    if kb == 0:
                        stk_ts = stK0_tiles
                    else:
                        stk_ts = []
                        for j in range(J):
                            st_t = stK_pool.tile([P, NMB, P], f16, tag="stK")
                            if kb == 1:
                                nc.scalar.dma_start(st_t[:], stK_d[j, kb])
                            else:
                                nc.sync.dma_start(st_t[:], stK_d[j, kb])
                            stk_ts.append(st_t)
                    stv = stv_pool.tile([P, CO], f16, tag="stv")
                    nc.scalar.dma_start(stv[:], stateK_d[kb])
                    cps = c_ps_pool.tile([P, CO], f32, tag="cps",
                                         name=f"cps{kb % 2}")
                    for mb in range(NMB):
                        for j in range(J):
                            nc.tensor.matmul(
                                cps[:],
                                stk_ts[j][:, mb, :],
                                y_tiles[j // 2][mb][:, (j % 2) * CO:
                                                    (j % 2 + 1) * CO],
                                start=(mb == 0 and j == 0),
                                stop=(mb == NMB - 1 and j == J - 1))
                    ct = cmb_pool.tile([P, CO], f16, tag="cmb",
                                       name=f"c{kb % 2}")
                    nc.scalar.activation(ct[:], cps[:], AF.Tanh)
                    # out = c + u*(state - c)
                    t1 = cmb_pool.tile([P, CO], f16, tag="cmb",
                                       name=f"t1_{kb % 2}")
                    nc.vector.tensor_sub(t1[:], stv[:], ct[:])
                    t2 = cmb_pool.tile([P, CO], f16, tag="cmb",
                                       name=f"t2_{kb % 2}")
                    nc.vector.tensor_mul(t2[:], uT_tiles[kb][:], t1[:])
                    t3 = cmb_pool.tile([P, CO], f16, tag="cmb",
                                       name=f"t3_{kb % 2}")
                    nc.vector.tensor_add(t3[:], ct[:], t2[:])
                    nc.scalar.dma_start(out_d[kb], t3[:])

    nc.compile()
    return nc


def _get_module():
    if "nc" not in _CACHE:
        _CACHE["nc"] = _build_module()
    return _CACHE["nc"]


def kernel(input, state, supports, Wr, br, Wu, bu, Wc, bc):
    input = np.asarray(input, np.float32)
    state = np.asarray(state, np.float32)
    supports = np.asarray(supports, np.float32)
    Wr = np.asarray(Wr, np.float32)
    br = np.asarray(br, np.float32)
    Wu = np.asarray(Wu, np.float32)
    bu = np.asarray(bu, np.float32)
    Wc = np.asarray(Wc, np.float32)
    bc = np.asarray(bc, np.float32)

    assert np.all(bc == 0.0), "kernel assumes bc == 0 (spec fill: zeros)"

    from concourse.bass_utils import run_bass_kernel_spmd

    nc = _get_module()

    f16 = np.float16
    st_host = np.ascontiguousarray(supports.transpose(0, 2, 1).astype(f16))
    stK_host = np.ascontiguousarray(
        st_host.reshape(J, NMB, P, NKB, P).transpose(0, 3, 2, 1, 4))
    wru = np.ascontiguousarray(np.concatenate([Wr, Wu], axis=2).astype(f16))
    # xc rows are [state(0:64), input(64:66)] on device; reorder Wc to match
    wc_r = np.concatenate([Wc[:, IN:, :], Wc[:, :IN, :]], axis=1)
    wc_host = np.ascontiguousarray(
        wc_r.reshape(J // 2, 2, F, OUT).transpose(0, 2, 1, 3)
        .reshape(J // 2, F, 2 * OUT).astype(f16))
    bru = np.concatenate([br, bu]).reshape(2 * OUT, 1).astype(np.float32)
    xs_full = np.concatenate([input, state], axis=2)  # [B, N, F]

    in_maps = []
    for c in range(NCORES):
        sl = slice(c * BC, (c + 1) * BC)
        xs_c = np.ascontiguousarray(
            xs_full[sl].transpose(1, 0, 2).reshape(N, CB).astype(f16))
        xinTg_c = np.ascontiguousarray(
            input[sl].astype(f16).reshape(BC, NG, GW, IN)
            .transpose(1, 3, 0, 2))
        st16 = state[sl].astype(f16)                   # [BC, N, OUT]
        stTg_c = np.ascontiguousarray(
            st16.reshape(BC, NG, GW, OUT).transpose(1, 3, 0, 2))
        stateK_c = np.ascontiguousarray(
            st16.reshape(BC, NKB, P, OUT).transpose(1, 2, 0, 3)
            .reshape(NKB, P, CO))
        in_maps.append({
            "st": st_host,
            "stK": stK_host,
            "xs": xs_c,
            "xinTg": xinTg_c,
            "stTg": stTg_c,
            "stateK": stateK_c,
            "wru": wru,
            "wc": wc_host,
            "bru": bru,
        })

    import time
    t0 = time.monotonic()
    res = run_bass_kernel_spmd(nc, in_maps, core_ids=list(range(NCORES)))
    _CACHE["last_wall_s"] = time.monotonic() - t0

    out = np.empty((B, N, OUT), np.float32)
    for c in range(NCORES):
        o2 = res.results[c]["out"]              # [NKB, P, BC*OUT] fp16
        o2 = o2.reshape(NKB, P, BC, OUT).transpose(2, 0, 1, 3)
        out[c * BC:(c + 1) * BC] = o2.reshape(BC, N, OUT).astype(np.float32)
    return out
